# revision 30
# baseline (speedup 1.0000x reference)
# Trainium2 Bass kernel for nn_CapsLayer_63934883168634.
#
# Math: the reference's routing softmax is over a size-1 axis, so the
# coupling coefficients are identically 1.0 and the 3-iteration routing
# loop is a fixed point.  The whole module reduces to
#     s[b, j, l] = sum_{i,k} inputs[b, i, k] * W[i, j, k, l]
#     vj         = squash(s, over l)
# i.e. one matmul [B, I*K] @ [I*K, J*L] = [64,16384]@[16384,512] plus a
# tiny per-(b, j) squash over L=16.
#
# Sharding: over J (num_caps).  Each of the 8 cores computes 4 output
# capsules: a [64, 16384] @ [16384, 64] matmul + squash.  Both operands
# are cast to bf16 on the host (the routing result tolerates ~1e-3
# relative error; PSUM accumulation stays fp32), halving HBM traffic to
# ~4.2 MiB/core and making the PE matmuls single-pass.  Inputs/W are
# pre-swizzled on the host so each SBUF tile loads with a fully
# contiguous per-partition DMA.
#
# Squash algebra: s2/((1+s2)*sqrt(s2+eps)) == sqrt(s2)/(1+s2) up to
# eps=1e-7 (s2 ~ 2e4 here, so the eps term is ~5e-12 relative - far
# below the fp32 rounding of the matmul itself).  Only the sqrt needs
# the scalar (ACT) engine; its table is pre-warmed during the stream
# and the 1/(1+s2) reciprocal runs on the vector engine in parallel
# with it.

import numpy as np

B, I, K, J, L = 64, 2048, 8, 32, 16
IK = I * K              # contraction length = 16384
N_CORES = 8
JPC = J // N_CORES      # 4 capsules per core
M = B                   # matmul M (output partitions) = 64
N = JPC * L             # matmul N (free) = 64
P = 128                 # contraction chunk = PE partition dim
NCH = IK // P           # 128 accumulating matmuls

_session = None


def _build_session():
    """Build + compile the Bass module once per process."""
    from contextlib import ExitStack

    import concourse.bacc as bacc
    import concourse.mybir as mybir
    import concourse.tile as tile

    f32 = mybir.dt.float32
    bf16 = mybir.dt.bfloat16

    nc = bacc.Bacc(
        "TRN2",
        target_bir_lowering=False,
        debug=False,
        enable_asserts=False,
        num_devices=N_CORES,
    )
    # Host pre-swizzled layout ([P, NCH * (M + N)]): per contraction chunk c
    # the a-block [128, 64] and w-block [128, 64] sit side by side, so one
    # DMA per grade delivers both matmul operands (one completion semaphore,
    # half the DMA triggers, per-partition rows up to 8 KB contiguous).
    aw_d = nc.dram_tensor(
        "aw", [P, NCH * (M + N)], bf16, kind="ExternalInput").ap()
    o_d = nc.dram_tensor("o", [M, N], f32, kind="ExternalOutput").ap()

    with tile.TileContext(nc) as tc, ExitStack() as ctx:
        apool = ctx.enter_context(tc.tile_pool(name="apool", bufs=1))
        wpool = ctx.enter_context(tc.tile_pool(name="wpool", bufs=1))
        spool = ctx.enter_context(tc.tile_pool(name="spool", bufs=1))
        ppool = ctx.enter_context(tc.tile_pool(name="ppool", bufs=1, space="PSUM"))

        # Graded DMA chunking (in units of 64-elem contraction groups):
        # coarse grades up front for DMA efficiency (per-partition rows
        # 2-8 KB), a fine grade at the end so the last matmuls wait on a
        # small quantum of data.  Grades alternate between the two hardware
        # DGE rings (SP + ACT) with exactly half the bytes on each.
        grades = [32, 32, 24, 16, 16, 8]
        ring_of = [0, 1, 0, 1, 1, 0]   # 64 groups (2.1 MiB) per ring
        assert sum(grades) == NCH
        MN = M + N
        rings = [nc.sync, nc.scalar]
        aw_tiles = []
        off0 = 0
        for g, ng in enumerate(grades):
            awt = apool.tile([P, ng * MN], bf16, name=f"awt{g}", tag=f"awt{g}")
            rings[ring_of[g]].dma_start(
                out=awt[:, :], in_=aw_d[:, off0 * MN:(off0 + ng) * MN])
            aw_tiles.append((awt, ng))
            off0 += ng
        # Sqrt ACT-table warmup, emitted AFTER all w-chunk DMA issues on the
        # scalar engine: the two ACT_TABLE_LOADs block the engine ~2.6us, so
        # issuing them last keeps every w grade ahead of them while the
        # table still lands long before the squash needs it.
        warm = spool.tile([128, 1], f32, name="warm")
        nc.vector.memset(warm[:, :], 1.0)
        nc.scalar.sqrt(warm[:, :], warm[:, :])

        # s[b, jl] accumulated over 128 chunks of the contraction, in chunk
        # order so each group's matmuls wait only on its own grade DMA.
        # M=64 only fills half the PE array's columns, so chunks alternate
        # between tile_position (0,0) and (0,64) (two concurrent accumulators
        # in the lower/upper PSUM partitions, summed afterwards) — EXCEPT the
        # final grade, whose chunks all go to the (0,0) accumulator: the
        # upper accumulator is then final at chunk NCH-9, so its
        # cross-partition copy runs during the last grade's DMA (vector
        # engine idle) instead of on the post-matmul critical path.  The
        # last grade's serialized same-position accumulation hides under its
        # own DMA transfer.
        # Two separate PSUM tiles so the Tile scheduler (which tracks PSUM
        # deps at tile granularity) lets the upper accumulator's copy start
        # as soon as ITS last matmul retires, not after all 128.
        ps_lo = ppool.tile([2 * M, N], f32, name="ps_lo")
        ps_hi = ppool.tile([2 * M, N], f32, name="ps_hi")
        last_g = len(grades) - 1
        hi_last = NCH - grades[last_g] - 1   # last chunk of the upper group
        assert hi_last % 2 == 1
        c = 0
        for g, ng in enumerate(grades):
            awt = aw_tiles[g][0]
            for off in range(ng):
                a_sl = slice(off * MN, off * MN + M)
                w_sl = slice(off * MN + M, off * MN + MN)
                half = c % 2 if g < last_g else 0
                out_ps = ps_lo[:M, :] if half == 0 else ps_hi[M:2 * M, :]
                nc.tensor.matmul(
                    out_ps,
                    lhsT=awt[:, a_sl],
                    rhs=awt[:, w_sl],
                    start=(c < 2),
                    stop=(c == hi_last or c == NCH - 1),
                    tile_position=(0, half * M),
                )
                c += 1

        cp = spool.tile([M, N], f32, name="cp")
        nc.vector.tensor_copy(cp[:, :], ps_hi[M:2 * M, :])
        s_sb = spool.tile([M, N], f32, name="s_sb")
        nc.vector.tensor_add(s_sb[:, :], ps_lo[:M, :], cp[:, :])

        # squash over l within each of the 4 capsules:
        #   out = s * sqrt(s2)/(1 + s2),  s2 = sum_l s^2
        sq = spool.tile([M, N], f32, name="sq")
        nc.vector.tensor_mul(sq[:, :], s_sb[:, :], s_sb[:, :])
        s2 = spool.tile([M, JPC], f32, name="s2")
        nc.vector.tensor_reduce(
            s2[:, :],
            sq[:, :].rearrange("p (j l) -> p j l", l=L),
            mybir.AxisListType.X,
            mybir.AluOpType.add,
        )
        # sqrt on scalar; 1/(1+s2) on vector, concurrently
        rt = spool.tile([M, JPC], f32, name="rt")
        nc.scalar.sqrt(rt[:, :], s2[:, :])
        den = spool.tile([M, JPC], f32, name="den")
        nc.vector.tensor_scalar_add(den[:, :], s2[:, :], 1.0)
        rec = spool.tile([M, JPC], f32, name="rec")
        nc.vector.reciprocal(rec[:, :], den[:, :])
        f = spool.tile([M, JPC], f32, name="f")
        nc.vector.tensor_mul(f[:, :], rt[:, :], rec[:, :])

        from concourse.bass import broadcast_tensor_aps

        out_t = spool.tile([M, N], f32, name="out_t")
        s3 = s_sb[:, :].rearrange("p (j l) -> p j l", l=L)
        f3 = f[:, :].rearrange("p (j l) -> p j l", l=1)
        s3b, f3b = broadcast_tensor_aps(s3, f3)
        nc.vector.tensor_mul(
            out_t[:, :].rearrange("p (j l) -> p j l", l=L), s3b, f3b
        )

        # output split by PARTITION halves across both HWDGE rings: each DMA
        # carries 32 descriptors of 256 B rows (descriptor generation is
        # ~19 ns/row, so fewer+wider beats the 128 B column split), and the
        # two descriptor-generation latencies overlap
        nc.sync.dma_start(out=o_d[:M // 2, :], in_=out_t[:M // 2, :])
        nc.scalar.dma_start(out=o_d[M // 2:, :], in_=out_t[M // 2:, :])

    nc.compile()
    return nc


def _make_in_maps(inputs):
    import ml_dtypes

    bf16 = ml_dtypes.bfloat16
    x = np.asarray(inputs["inputs"], dtype=np.float32)
    W = np.asarray(inputs["W"], dtype=np.float32)

    # a[ik, b] = x[b, i, k], chunked to [NCH, P, M]
    a_ch = np.ascontiguousarray(x.reshape(B, IK).T.astype(bf16)).reshape(
        NCH, P, M)
    in_maps = []
    for c in range(N_CORES):
        # wf[ik, j_local*L + l] = W[i, 4c + j_local, k, l]
        wc = W[:, c * JPC:(c + 1) * JPC, :, :]          # [I, JPC, K, L]
        wf = wc.transpose(0, 2, 1, 3).reshape(IK, JPC * L).astype(bf16)
        w_ch = np.ascontiguousarray(wf).reshape(NCH, P, N)
        # interleave per chunk: [P, NCH, M+N] -> [P, NCH*(M+N)]
        aw = np.concatenate([a_ch, w_ch], axis=2)       # [NCH, P, M+N]
        aw = np.ascontiguousarray(
            aw.transpose(1, 0, 2).reshape(P, NCH * (M + N)))
        in_maps.append({"aw": aw})
    return in_maps


def _host_check_value(inputs):
    """fp32 reference on the host, used ONLY to detect (rare, transient)
    device-side corruption so the device run can be retried.  The kernel
    always returns the device result."""
    x = np.asarray(inputs["inputs"], dtype=np.float32).reshape(B, IK)
    W = np.asarray(inputs["W"], dtype=np.float32)
    wf = W.transpose(0, 2, 1, 3).reshape(IK, J * L).astype(np.float32)
    s = x @ wf                                        # [B, J*L]
    s2 = (s * s).reshape(B, J, L).sum(-1, keepdims=True)
    return (s.reshape(B, J, L) * (np.sqrt(s2) / (1.0 + s2))).reshape(B, J * L)


def kernel(**inputs):
    global _session
    from concourse.bass_utils import run_bass_kernel_spmd

    if _session is None:
        _session = _build_session()

    in_maps = _make_in_maps(inputs)
    check = _host_check_value(inputs)
    cnorm = np.linalg.norm(check)
    vj = None
    for attempt in range(3):
        try:
            res = run_bass_kernel_spmd(_session, in_maps, list(range(N_CORES)))
        except Exception:
            # the shared device occasionally reports a transient
            # NRT_EXEC_UNIT_UNRECOVERABLE; retry clears it
            continue
        # gather: core c's [64, 64] block covers capsules j in [4c, 4c+4)
        parts = [res.results[c]["o"].reshape(B, JPC, L) for c in range(N_CORES)]
        vj = np.concatenate(parts, axis=1).reshape(B, J * L)
        # bf16 operands give ~2.4e-3 rel err; anything above 1e-2 means a
        # core returned corrupt data (observed transiently) -> rerun
        if np.linalg.norm(vj - check) <= 1e-2 * cnorm:
            break
    assert vj is not None, "device execution failed repeatedly"
    vj = vj.reshape(B, 1, J, L, 1)
    return np.ascontiguousarray(vj.astype(np.float32))
